# revision 1
# baseline (speedup 1.0000x reference)
"""Trainium2 Bass kernel for a 3D-gaussian-splatting rasterizer.

Pipeline:
  host (numpy, O(N) work): quaternion -> cov3D -> EWA cov2D -> conic,
    projection, depth sort, per-cell culling (8x16 half-tiles, with a
    16x16 merge when the halves share most gaussians), packing of
    (cell, gaussian-chunk) slabs into [127-row x 256-col] bins.
  device (8 NeuronCores, SPMD), per bin, fully independent (no carry
    chain): power = coef-matmul over an 8-term basis (6 quadratic terms
    + 2 slot indicators); alpha = exp(power); L = ln(1-alpha);
    cum = strict-triangular matmul accumulated ONTO the power PSUM so
    S = power + cumsum(L); w = exp(S) = alpha * T_excl; rgb matmul.
    Row 127 is reserved: its power is -0.5 so exp(S[127]) = e^-.5 * T_bin
    gives the per-pixel bin transmittance for free.
  host: combine slabs of multi-chunk cells front-to-back
    (rgb += T_prefix * rgb_slab), add residual transmittance * bg,
    scatter cells into the [3,128,128] image.

No per-element masking is applied (the reference zeroes alpha < 1/255);
numerically validated: rel err ~6e-3 vs the 2e-2 gate.
"""

import os
import numpy as np

N_CORES = 8
H = W = 128
TS = 16            # tile (full basis pattern) is 16x16
PIX = 256          # columns per bin
ROWCAP = 127       # gaussian rows per bin (row 127 reserved)
KB = 128
TANFOV = 0.5
FOCAL = W / (2.0 * TANFOV)   # 128.0
ZNEAR = 0.2
ALPHA_MIN = 1.0 / 255.0
NEG_BIG = -1.0e9
RESV = -0.5        # reserved-row power; T = otr * e^{-RESV}

_compiled_cache = {}


# ----------------------------------------------------------------------------
# Host-side per-gaussian preprocessing (numpy, O(N))
# ----------------------------------------------------------------------------

def _preprocess(means3D, opacities, colors_precomp, scales, rotations, viewmatrix):
    q = rotations / np.linalg.norm(rotations, axis=-1, keepdims=True)
    r, x, y, z = q[:, 0], q[:, 1], q[:, 2], q[:, 3]
    R = np.stack([
        1 - 2 * (y * y + z * z), 2 * (x * y - r * z), 2 * (x * z + r * y),
        2 * (x * y + r * z), 1 - 2 * (x * x + z * z), 2 * (y * z - r * x),
        2 * (x * z - r * y), 2 * (y * z + r * x), 1 - 2 * (x * x + y * y),
    ], axis=-1).reshape(-1, 3, 3)
    M = R * scales[:, None, :]
    cov3D = np.einsum('nij,nkj->nik', M, M)

    Wm = viewmatrix[:3, :3]
    t = means3D @ Wm.T + viewmatrix[:3, 3]
    tz = t[:, 2]
    lim = 1.3 * TANFOV
    txz = np.clip(t[:, 0] / tz, -lim, lim) * tz
    tyz = np.clip(t[:, 1] / tz, -lim, lim) * tz
    zero = np.zeros_like(tz)
    fx = fy = FOCAL
    J = np.stack([
        np.stack([fx / tz, zero, -fx * txz / (tz * tz)], axis=-1),
        np.stack([zero, fy / tz, -fy * tyz / (tz * tz)], axis=-1),
    ], axis=1)
    T = np.einsum('nij,jk->nik', J, Wm)
    cov2D = np.einsum('nij,njk,nlk->nil', T, cov3D, T)
    a = cov2D[:, 0, 0] + 0.3
    b = cov2D[:, 0, 1]
    c = cov2D[:, 1, 1] + 0.3
    det = a * c - b * b
    det_safe = np.where(det > 0, det, 1.0)
    conA, conB, conC = c / det_safe, -b / det_safe, a / det_safe
    px = fx * t[:, 0] / tz + (W - 1) * 0.5
    py = fy * t[:, 1] / tz + (H - 1) * 0.5
    valid = (det > 0) & (tz > ZNEAR)
    opac = opacities[:, 0]

    # bounding half-widths of the {alpha >= ALPHA_MIN} ellipse
    ell = np.log(np.maximum(opac * 255.0, 1.0 + 1e-7))
    rx = np.where(valid, np.sqrt(np.maximum(2 * ell * a, 0.0)), 0.0)
    ry = np.where(valid, np.sqrt(np.maximum(2 * ell * c, 0.0)), 0.0)

    order = np.argsort(tz, kind='stable')
    d = dict(conA=conA, conB=conB, conC=conC, px=px, py=py, opac=opac,
             cols=colors_precomp, valid=valid, rx=rx, ry=ry, ell=ell)
    return {k: (v[order] if k != 'cols' else v[order]) for k, v in d.items()}


def _cull_rect(pre, xlo, ylo, w, h):
    """Indices (depth-ordered) of gaussians touching rect, ellipse-corner
    refined."""
    px, py, rx, ry = pre['px'], pre['py'], pre['rx'], pre['ry']
    xhi, yhi = xlo + w - 1, ylo + h - 1
    hit = pre['valid'] & (px + rx >= xlo) & (px - rx <= xhi) \
        & (py + ry >= ylo) & (py - ry <= yhi)
    cx = np.clip(px, xlo, xhi)
    cy = np.clip(py, ylo, yhi)
    dx = cx - px
    dy = cy - py
    beyond = (dx != 0) & (dy != 0)
    quad = pre['conA'] * dx * dx + 2 * pre['conB'] * dx * dy \
        + pre['conC'] * dy * dy
    hit &= ~beyond | (quad <= 2 * pre['ell'])
    return np.nonzero(hit)[0]


def _build_slabs(pre, chunk_cap):
    """Per 16x16 tile choose halves (2 x 8x16) or full tile, chunk lists to
    <=chunk_cap rows, return slab dicts."""
    slabs = []   # dict(rect, idx, key, pos)
    for ti in range(H // TS):
        for tj in range(W // TS):
            xlo, ylo = tj * TS, ti * TS
            idx_l = _cull_rect(pre, xlo, ylo, 8, 16)
            idx_r = _cull_rect(pre, xlo + 8, ylo, 8, 16)
            nl, nr = len(idx_l), len(idx_r)
            if nl == 0 and nr == 0:
                continue
            # full-tile merge when it saves rows and fits one bin
            idx_f = _cull_rect(pre, xlo, ylo, 16, 16)
            nf = len(idx_f)
            use_full = nf <= ROWCAP and nl + nr >= 1.30 * nf and nf > 0
            if use_full:
                slabs.append(dict(rect=(xlo, ylo, 16, 16), idx=idx_f,
                                  key=(ti, tj, 'f'), pos=0, npos=1))
            else:
                for sx, idx in ((0, idx_l), (1, idx_r)):
                    n = len(idx)
                    if n == 0:
                        continue
                    k = -(-n // chunk_cap)
                    chunks = np.array_split(idx, k)
                    for s, ch in enumerate(chunks):
                        slabs.append(dict(rect=(xlo + 8 * sx, ylo, 8, 16),
                                          idx=ch, key=(ti, tj, sx),
                                          pos=s, npos=k))
    return slabs


def _pack_bins_bfd(slabs):
    """Best-fit-decreasing: bins hold <=256 cols (full=256, half=128) and
    <=ROWCAP rows."""
    items = sorted(slabs, key=lambda s: -len(s['idx']))
    bins = []        # list of lists
    space = []       # (cols_left, rows_left)
    for s in items:
        cols = 256 if s['rect'][2] == 16 else 128
        rows = len(s['idx'])
        best, best_slack = -1, None
        for i, (cl, rl) in enumerate(space):
            if cl >= cols and rl >= rows:
                slack = rl - rows
                if best < 0 or slack < best_slack:
                    best, best_slack = i, slack
        if best < 0:
            bins.append([s])
            space.append((256 - cols, ROWCAP - rows))
        else:
            bins[best].append(s)
            cl, rl = space[best]
            space[best] = (cl - cols, rl - rows)
    return bins


def _pack_bins(pre):
    best = None
    for cap in (127, 96, 85, 64):
        slabs = _build_slabs(pre, cap)
        bins = _pack_bins_bfd(slabs)
        if best is None or len(bins) < len(best):
            best = bins
    return best


def _make_basis():
    """[8, 256]: rows x^2,y^2,xy,x,y,1,ind0,ind1. Cols 0-127: left half of a
    16x16 tile (x_rel -7.5..-0.5), 128-255: right half; y-major within."""
    basis = np.zeros((8, 256), np.float32)
    for s in range(2):
        for yy in range(16):
            for xx in range(8):
                c = s * 128 + yy * 8 + xx
                xr = xx + 8 * s - 7.5
                yr = yy - 7.5
                basis[:, c] = [xr * xr, yr * yr, xr * yr, xr, yr, 1.0,
                               1.0 - s, float(s)]
    return basis


def _build_core_arrays(pre, core_bins, NB):
    """coef [8, NB*128] f32, colsT [128, 4*NB] f32 for one core.
    Returns also slab placement records."""
    coef = np.zeros((8, NB * KB), np.float32)
    colsT = np.zeros((KB, 4 * NB), np.float32)
    # default: padding rows (power = -BIG via const row x basis row5..7)
    coef[5, :] = NEG_BIG
    placements = []
    conA, conB, conC = pre['conA'], pre['conB'], pre['conC']
    px, py, opac, cols = pre['px'], pre['py'], pre['opac'], pre['cols']
    for b, bin_slabs in enumerate(core_bins):
        row = 0
        colbase = 0
        for slab in bin_slabs:
            xlo, ylo, w, hgt = slab['rect']
            idx = slab['idx']
            n = len(idx)
            # slot assignment: full tile uses both slots, half uses one
            if w == 16:
                slot = (0, 1)
                vx = xlo + 7.5
                ncols = 256
            else:
                slot = (colbase // 128,)
                vx = xlo + 7.5 - 8 * (colbase // 128)
                ncols = 128
            vy = ylo + 7.5
            A, Bc, C = conA[idx], conB[idx], conC[idx]
            pxr = px[idx] - vx
            pyr = py[idx] - vy
            sl = slice(b * KB + row, b * KB + row + n)
            coef[0, sl] = -0.5 * A
            coef[1, sl] = -0.5 * C
            coef[2, sl] = -Bc
            coef[3, sl] = A * pxr + Bc * pyr
            coef[4, sl] = C * pyr + Bc * pxr
            coef[5, sl] = -0.5 * (A * pxr * pxr + C * pyr * pyr) \
                - Bc * pxr * pyr + np.log(opac[idx])
            coef[6, sl] = 0.0 if 0 in slot else NEG_BIG
            coef[7, sl] = 0.0 if 1 in slot else NEG_BIG
            colsT[row:row + n, b * 4:b * 4 + 3] = cols[idx]
            placements.append(dict(key=slab['key'], pos=slab['pos'],
                                   npos=slab['npos'], rect=slab['rect'],
                                   bin=b, col0=colbase, ncols=ncols))
            row += n
            colbase += ncols
        # reserved transmittance row
        coef[:, b * KB + ROWCAP] = [0, 0, 0, 0, 0, RESV, 0, 0]
    return coef, colsT, placements


# ----------------------------------------------------------------------------
# Device program
# ----------------------------------------------------------------------------

def _build_program(NB):
    from contextlib import ExitStack
    import concourse.bass as bass  # noqa: F401
    import concourse.tile as tile
    from concourse import mybir, bacc

    f32 = mybir.dt.float32
    f32r = mybir.dt.float32r
    bf16 = mybir.dt.bfloat16
    AF = mybir.ActivationFunctionType

    class _BaccOneActSet(bacc.Bacc):
        # Pin Exp/Ln to the one table set containing both, so the scalar
        # engine loads activation tables once.
        def insert_act_table_loads(self):
            from concourse.hw_specs import get_activation_tables
            from concourse.bacc import _bass_rust
            tables = []
            for name, fns in get_activation_tables(self.m.arch).items():
                if name != 'natural_log_exp_and_others':
                    fns = fns - {AF.Exp, AF.Ln}
                tables.append((name, fns))
            _bass_rust.insert_act_table_loads(self, tables)

    nc = _BaccOneActSet(None)
    coef_d = nc.declare_dram_parameter("coef", [8, NB * KB], f32r,
                                       isOutput=False)
    basis_d = nc.declare_dram_parameter("basis", [8, PIX], f32r,
                                        isOutput=False)
    u_d = nc.declare_dram_parameter("u", [KB, KB], bf16, isOutput=False)
    cols_d = nc.declare_dram_parameter("cols", [KB, 4 * NB], bf16,
                                       isOutput=False)
    orgb_d = nc.declare_dram_parameter("orgb", [3, NB * PIX], f32,
                                       isOutput=True)
    otr_d = nc.declare_dram_parameter("otr", [1, NB * PIX], f32,
                                      isOutput=True)

    groups = [(g, min(g + 2, NB)) for g in range(0, NB, 2)]

    with ExitStack() as ctx:
        tc = ctx.enter_context(tile.TileContext(
            nc, linearize=bool(int(os.environ.get("GR_LINEARIZE", "0")))))
        const_pool = ctx.enter_context(tc.tile_pool(name="const", bufs=1))
        sb = ctx.enter_context(tc.tile_pool(name="work", bufs=2))
        ps = ctx.enter_context(tc.tile_pool(name="psum", bufs=2, space="PSUM"))

        coef_sb = const_pool.tile([8, NB * KB], f32r)
        basis_sb = const_pool.tile([8, PIX], f32r)
        u_sb = const_pool.tile([KB, KB], bf16)
        cols_sb = const_pool.tile([KB, 4 * NB], bf16)

        nc.gpsimd.dma_start(coef_sb[:], coef_d[:])
        nc.gpsimd.dma_start(basis_sb[:], basis_d[:])
        nc.gpsimd.dma_start(u_sb[:], u_d[:])
        nc.gpsimd.dma_start(cols_sb[:], cols_d[:])

        for gi, (g0, g1) in enumerate(groups):
            nb = g1 - g0
            FD = nb * PIX
            P = ps.tile([KB, 512], f32, tag="p")
            for i in range(nb):
                b = g0 + i
                nc.tensor.matmul(
                    P[:, i * PIX:(i + 1) * PIX],
                    lhsT=coef_sb[:, b * KB:(b + 1) * KB],
                    rhs=basis_sb,
                    start=True, stop=True)
            A = sb.tile([KB, 512], f32, tag="alpha")
            nc.scalar.activation(A[:, :FD], P[:, :FD], AF.Exp)
            L = sb.tile([KB, 512], bf16, tag="lnT")
            nc.scalar.activation(L[:, :FD], A[:, :FD], AF.Ln,
                                 bias=1.0, scale=-1.0)
            C = ps.tile([KB, 512], f32, tag="cum")
            for i in range(nb):
                sl = slice(i * PIX, (i + 1) * PIX)
                nc.tensor.matmul(C[:, sl], lhsT=u_sb, rhs=L[:, sl],
                                 start=True, stop=True)
            T = sb.tile([KB, 512], f32, tag="trans")
            nc.scalar.activation(T[:, :FD], C[:, :FD], AF.Exp)
            Wt = sb.tile([KB, 512], bf16, tag="wgt")
            nc.gpsimd.tensor_mul(Wt[:, :FD], T[:, :FD], A[:, :FD])
            R = ps.tile([4, 512], f32, tag="rgb")
            for i in range(nb):
                b = g0 + i
                sl = slice(i * PIX, (i + 1) * PIX)
                nc.tensor.matmul(R[:, sl],
                                 lhsT=cols_sb[:, b * 4:(b + 1) * 4],
                                 rhs=Wt[:, sl],
                                 start=True, stop=True)
            Rsb = sb.tile([3, 512], f32, tag="rgbsb")
            nc.vector.tensor_copy(Rsb[:, :FD], R[0:3, :FD])
            nc.gpsimd.dma_start(orgb_d[:, g0 * PIX:g0 * PIX + FD],
                                Rsb[:, :FD])
            nc.gpsimd.dma_start(otr_d[:, g0 * PIX:g0 * PIX + FD],
                                T[ROWCAP:ROWCAP + 1, :FD])

    nc.compile()
    return nc


# ----------------------------------------------------------------------------
# Entry point
# ----------------------------------------------------------------------------

def kernel(means3D, means2D, opacities, colors_precomp, scales, rotations,
           bg, viewmatrix):
    import ml_dtypes
    means3D = np.asarray(means3D, np.float32)
    opacities = np.asarray(opacities, np.float32)
    colors_precomp = np.asarray(colors_precomp, np.float32)
    scales = np.asarray(scales, np.float32)
    rotations = np.asarray(rotations, np.float32)
    bg = np.asarray(bg, np.float32)
    viewmatrix = np.asarray(viewmatrix, np.float32)

    pre = _preprocess(means3D, opacities, colors_precomp, scales, rotations,
                      viewmatrix)
    bins = _pack_bins(pre)
    nbins = len(bins)
    NB = max(1, -(-nbins // N_CORES))
    if bool(int(os.environ.get("GR_DEBUG", "0"))):
        rows = sum(len(s['idx']) for b in bins for s in b)
        print(f"[gr] bins={nbins} NB={NB} rows={rows}")

    core_bins = [bins[c::N_CORES] for c in range(N_CORES)]
    basis = _make_basis()
    ustrict = np.triu(np.ones((KB, KB), np.float32), 1)

    in_maps = []
    all_placements = []
    for core in range(N_CORES):
        coef, colsT, placements = _build_core_arrays(pre, core_bins[core], NB)
        all_placements.append(placements)
        in_maps.append(dict(
            coef=coef,
            basis=basis,
            u=ustrict.astype(ml_dtypes.bfloat16),
            cols=colsT.astype(ml_dtypes.bfloat16)))

    if NB not in _compiled_cache:
        _compiled_cache[NB] = _build_program(NB)
    nc = _compiled_cache[NB]

    from concourse.bass_utils import run_bass_kernel_spmd
    trace = bool(int(os.environ.get("GR_TRACE", "0")))
    res = run_bass_kernel_spmd(nc, in_maps, list(range(N_CORES)), trace=trace)
    if trace:
        kernel.last_exec_time_ns = res.exec_time_ns
        kernel.last_profile = res.profile_json

    # ---- host combine ----
    out = np.zeros((3, H, W), np.float32) + bg[:, None, None]
    tresc = 1.0
    chains = {}
    for core in range(N_CORES):
        orgb = res.results[core]["orgb"]
        otr = np.asarray(res.results[core]["otr"], np.float32) * tresc
        for pl in all_placements[core]:
            c0 = pl['bin'] * PIX + pl['col0']
            rgb = orgb[:, c0:c0 + pl['ncols']]
            T = otr[0, c0:c0 + pl['ncols']]
            chains.setdefault(pl['key'], []).append(
                (pl['pos'], pl['rect'], rgb, T, pl['npos']))
    for key, parts in chains.items():
        parts.sort(key=lambda p: p[0])
        _, rect, rgb0, T0, _ = parts[0]
        acc = rgb0.astype(np.float32).copy()
        Tacc = T0.copy()
        for _, _, rgb, T, _ in parts[1:]:
            acc += Tacc[None, :] * rgb
            Tacc = Tacc * T
        acc += Tacc[None, :] * bg[:, None]
        xlo, ylo, w, hgt = rect
        if w == 16:
            left = acc[:, 0:128].reshape(3, 16, 8)
            right = acc[:, 128:256].reshape(3, 16, 8)
            out[:, ylo:ylo + 16, xlo:xlo + 8] = left
            out[:, ylo:ylo + 16, xlo + 8:xlo + 16] = right
        else:
            out[:, ylo:ylo + hgt, xlo:xlo + w] = acc.reshape(3, hgt, w)
    return out



# revision 3
# speedup vs baseline: 1.2993x; 1.2993x over previous
"""Trainium2 Bass kernel for a 3D-gaussian-splatting rasterizer.

Pipeline:
  host (numpy, O(N) work): quaternion -> cov3D -> EWA cov2D -> conic,
    projection, depth sort, per-cell culling (8x16 half-tiles, with a
    16x16 merge when the halves share most gaussians), packing of
    (cell, gaussian-chunk) slabs into [127-row x 256-col] bins.
  device (8 NeuronCores, SPMD), iteration i = bin pair (2i, 2i+1),
    512 pixel-columns:
      P   = coef-matmul over a 16-term stacked basis (one matmul per
            bin PAIR: rows 0-7 drive cols 0-255, rows 8-15 cols 256-511)
      A   = exp(P)                  (scalar engine)
      L   = ln(1 - A)               (scalar engine, bf16)
      P  += Ustrict @ L             (matmul ACCUMULATED onto the power
            PSUM, start=False -> S = power + cumsum_strict(L))
      W   = exp(S) = alpha * T_excl (scalar engine, bf16) -- the
            compositing weights directly, no elementwise multiply.
      R   = colsT @ W               (per-bin rgb matmul; color column 3
            holds e^{+0.5} at reserved row 127, so R[3] = T_bin because
            W[127] = e^{-0.5} * T_bin)
      copy R -> SBUF (vector engine, bf16), per-iter DMA out on the
      sync engine (hardware DGE; gpsimd software DGE is slow).
    Emission is software-pipelined so the scalar engine (bottleneck)
    never waits on the tensor engine.
  host: combine slabs of multi-chunk cells front-to-back
    (rgb += T_prefix * rgb_slab), add residual transmittance * bg,
    scatter cells into the [3,128,128] image.

No per-element masking is applied (the reference zeroes alpha < 1/255);
numerically validated vs the 2e-2 gate.
"""

import os
import numpy as np

N_CORES = 8
H = W = 128
TS = 16            # tile (full basis pattern) is 16x16
PIX = 256          # columns per bin
ROWCAP = 127       # gaussian rows per bin (row 127 reserved)
KB = 128
TANFOV = 0.5
FOCAL = W / (2.0 * TANFOV)   # 128.0
ZNEAR = 0.2
ALPHA_MIN = 1.0 / 255.0
NEG_BIG = -1.0e9
RESV = -0.5        # reserved-row power; W[127] = e^{RESV} * T_bin

_compiled_cache = {}


# ----------------------------------------------------------------------------
# Host-side per-gaussian preprocessing (numpy, O(N))
# ----------------------------------------------------------------------------

def _preprocess(means3D, opacities, colors_precomp, scales, rotations, viewmatrix):
    q = rotations / np.linalg.norm(rotations, axis=-1, keepdims=True)
    r, x, y, z = q[:, 0], q[:, 1], q[:, 2], q[:, 3]
    R = np.stack([
        1 - 2 * (y * y + z * z), 2 * (x * y - r * z), 2 * (x * z + r * y),
        2 * (x * y + r * z), 1 - 2 * (x * x + z * z), 2 * (y * z - r * x),
        2 * (x * z - r * y), 2 * (y * z + r * x), 1 - 2 * (x * x + y * y),
    ], axis=-1).reshape(-1, 3, 3)
    M = R * scales[:, None, :]
    cov3D = np.einsum('nij,nkj->nik', M, M)

    Wm = viewmatrix[:3, :3]
    t = means3D @ Wm.T + viewmatrix[:3, 3]
    tz = t[:, 2]
    lim = 1.3 * TANFOV
    txz = np.clip(t[:, 0] / tz, -lim, lim) * tz
    tyz = np.clip(t[:, 1] / tz, -lim, lim) * tz
    zero = np.zeros_like(tz)
    fx = fy = FOCAL
    J = np.stack([
        np.stack([fx / tz, zero, -fx * txz / (tz * tz)], axis=-1),
        np.stack([zero, fy / tz, -fy * tyz / (tz * tz)], axis=-1),
    ], axis=1)
    T = np.einsum('nij,jk->nik', J, Wm)
    cov2D = np.einsum('nij,njk,nlk->nil', T, cov3D, T)
    a = cov2D[:, 0, 0] + 0.3
    b = cov2D[:, 0, 1]
    c = cov2D[:, 1, 1] + 0.3
    det = a * c - b * b
    det_safe = np.where(det > 0, det, 1.0)
    conA, conB, conC = c / det_safe, -b / det_safe, a / det_safe
    px = fx * t[:, 0] / tz + (W - 1) * 0.5
    py = fy * t[:, 1] / tz + (H - 1) * 0.5
    valid = (det > 0) & (tz > ZNEAR)
    opac = opacities[:, 0]

    # bounding half-widths of the {alpha >= ALPHA_MIN} ellipse
    ell = np.log(np.maximum(opac * 255.0, 1.0 + 1e-7))
    rx = np.where(valid, np.sqrt(np.maximum(2 * ell * a, 0.0)), 0.0)
    ry = np.where(valid, np.sqrt(np.maximum(2 * ell * c, 0.0)), 0.0)

    order = np.argsort(tz, kind='stable')
    d = dict(conA=conA, conB=conB, conC=conC, px=px, py=py, opac=opac,
             cols=colors_precomp, valid=valid, rx=rx, ry=ry, ell=ell)
    return {k: (v[order] if k != 'cols' else v[order]) for k, v in d.items()}


def _cull_rect(pre, xlo, ylo, w, h):
    """Indices (depth-ordered) of gaussians touching rect, ellipse-corner
    refined."""
    px, py, rx, ry = pre['px'], pre['py'], pre['rx'], pre['ry']
    xhi, yhi = xlo + w - 1, ylo + h - 1
    hit = pre['valid'] & (px + rx >= xlo) & (px - rx <= xhi) \
        & (py + ry >= ylo) & (py - ry <= yhi)
    cx = np.clip(px, xlo, xhi)
    cy = np.clip(py, ylo, yhi)
    dx = cx - px
    dy = cy - py
    beyond = (dx != 0) & (dy != 0)
    quad = pre['conA'] * dx * dx + 2 * pre['conB'] * dx * dy \
        + pre['conC'] * dy * dy
    hit &= ~beyond | (quad <= 2 * pre['ell'])
    return np.nonzero(hit)[0]


def _build_slabs(pre, chunk_cap):
    """Per 16x16 tile choose halves (2 x 8x16) or full tile, chunk lists to
    <=chunk_cap rows, return slab dicts."""
    slabs = []   # dict(rect, idx, key, pos)
    for ti in range(H // TS):
        for tj in range(W // TS):
            xlo, ylo = tj * TS, ti * TS
            idx_l = _cull_rect(pre, xlo, ylo, 8, 16)
            idx_r = _cull_rect(pre, xlo + 8, ylo, 8, 16)
            nl, nr = len(idx_l), len(idx_r)
            if nl == 0 and nr == 0:
                continue
            # full-tile merge when it saves rows and fits one bin
            idx_f = _cull_rect(pre, xlo, ylo, 16, 16)
            nf = len(idx_f)
            use_full = nf <= ROWCAP and nl + nr >= 1.30 * nf and nf > 0
            if use_full:
                slabs.append(dict(rect=(xlo, ylo, 16, 16), idx=idx_f,
                                  key=(ti, tj, 'f'), pos=0, npos=1))
            else:
                for sx, idx in ((0, idx_l), (1, idx_r)):
                    n = len(idx)
                    if n == 0:
                        continue
                    k = -(-n // chunk_cap)
                    chunks = np.array_split(idx, k)
                    for s, ch in enumerate(chunks):
                        slabs.append(dict(rect=(xlo + 8 * sx, ylo, 8, 16),
                                          idx=ch, key=(ti, tj, sx),
                                          pos=s, npos=k))
    return slabs


def _pack_bins_bfd(slabs):
    """Best-fit-decreasing: bins hold <=256 cols (full=256, half=128) and
    <=ROWCAP rows."""
    items = sorted(slabs, key=lambda s: -len(s['idx']))
    bins = []        # list of lists
    space = []       # (cols_left, rows_left)
    for s in items:
        cols = 256 if s['rect'][2] == 16 else 128
        rows = len(s['idx'])
        best, best_slack = -1, None
        for i, (cl, rl) in enumerate(space):
            if cl >= cols and rl >= rows:
                slack = rl - rows
                if best < 0 or slack < best_slack:
                    best, best_slack = i, slack
        if best < 0:
            bins.append([s])
            space.append((256 - cols, ROWCAP - rows))
        else:
            bins[best].append(s)
            cl, rl = space[best]
            space[best] = (cl - cols, rl - rows)
    return bins


def _pack_bins(pre):
    best = None
    for cap in (127, 96, 85, 64):
        slabs = _build_slabs(pre, cap)
        bins = _pack_bins_bfd(slabs)
        if best is None or len(bins) < len(best):
            best = bins
    return best


def _make_basis():
    """[8, 256]: rows x^2,y^2,xy,x,y,1,ind0,ind1. Cols 0-127: left half of a
    16x16 tile (x_rel -7.5..-0.5), 128-255: right half; y-major within."""
    basis = np.zeros((8, 256), np.float32)
    for s in range(2):
        for yy in range(16):
            for xx in range(8):
                c = s * 128 + yy * 8 + xx
                xr = xx + 8 * s - 7.5
                yr = yy - 7.5
                basis[:, c] = [xr * xr, yr * yr, xr * yr, xr, yr, 1.0,
                               1.0 - s, float(s)]
    return basis


def _build_core_arrays(pre, core_bins, NB):
    """coef [8, NB*128] f32, colsT [128, 4*NB] f32 for one core.
    colsT column 3 carries e^{+0.5} at reserved row 127 so the rgb matmul
    emits the bin transmittance in output row 3.
    Returns also slab placement records."""
    coef = np.zeros((8, NB * KB), np.float32)
    colsT = np.zeros((KB, 4 * NB), np.float32)
    # default: padding rows (power = -BIG via const row x basis row5..7)
    coef[5, :] = NEG_BIG
    placements = []
    conA, conB, conC = pre['conA'], pre['conB'], pre['conC']
    px, py, opac, cols = pre['px'], pre['py'], pre['opac'], pre['cols']
    for b, bin_slabs in enumerate(core_bins):
        row = 0
        colbase = 0
        for slab in bin_slabs:
            xlo, ylo, w, hgt = slab['rect']
            idx = slab['idx']
            n = len(idx)
            # slot assignment: full tile uses both slots, half uses one
            if w == 16:
                slot = (0, 1)
                vx = xlo + 7.5
                ncols = 256
            else:
                slot = (colbase // 128,)
                vx = xlo + 7.5 - 8 * (colbase // 128)
                ncols = 128
            vy = ylo + 7.5
            A, Bc, C = conA[idx], conB[idx], conC[idx]
            pxr = px[idx] - vx
            pyr = py[idx] - vy
            sl = slice(b * KB + row, b * KB + row + n)
            coef[0, sl] = -0.5 * A
            coef[1, sl] = -0.5 * C
            coef[2, sl] = -Bc
            coef[3, sl] = A * pxr + Bc * pyr
            coef[4, sl] = C * pyr + Bc * pxr
            coef[5, sl] = -0.5 * (A * pxr * pxr + C * pyr * pyr) \
                - Bc * pxr * pyr + np.log(opac[idx])
            coef[6, sl] = 0.0 if 0 in slot else NEG_BIG
            coef[7, sl] = 0.0 if 1 in slot else NEG_BIG
            colsT[row:row + n, b * 4:b * 4 + 3] = cols[idx]
            placements.append(dict(key=slab['key'], pos=slab['pos'],
                                   npos=slab['npos'], rect=slab['rect'],
                                   bin=b, col0=colbase, ncols=ncols))
            row += n
            colbase += ncols
        # reserved transmittance row
        coef[:, b * KB + ROWCAP] = [0, 0, 0, 0, 0, RESV, 0, 0]
        colsT[ROWCAP, b * 4 + 3] = np.exp(0.5)
    return coef, colsT, placements


# ----------------------------------------------------------------------------
# Device program
# ----------------------------------------------------------------------------

def _build_program(NB):
    """NB must be even. Iteration i covers bins (2i, 2i+1) = 512 columns."""
    from contextlib import ExitStack
    import concourse.bass as bass  # noqa: F401
    import concourse.tile as tile
    from concourse import mybir, bacc

    f32 = mybir.dt.float32
    f32r = mybir.dt.float32r
    bf16 = mybir.dt.bfloat16
    AF = mybir.ActivationFunctionType

    G = NB // 2          # bin pairs == iterations
    FD = 512             # columns per iteration

    class _BaccOneActSet(bacc.Bacc):
        # Pin Exp/Ln to the one table set containing both, so the scalar
        # engine loads activation tables once.
        def insert_act_table_loads(self):
            from concourse.hw_specs import get_activation_tables
            from concourse.bacc import _bass_rust
            tables = []
            for name, fns in get_activation_tables(self.m.arch).items():
                if name != 'natural_log_exp_and_others':
                    fns = fns - {AF.Exp, AF.Ln}
                tables.append((name, fns))
            _bass_rust.insert_act_table_loads(self, tables)

    nc = _BaccOneActSet(None)
    coef_d = nc.declare_dram_parameter("coef2", [16, G * KB], f32r,
                                       isOutput=False)
    basis_d = nc.declare_dram_parameter("basis2", [16, FD], f32r,
                                        isOutput=False)
    ucols_d = nc.declare_dram_parameter("ucols", [KB, KB + 4 * NB], bf16,
                                        isOutput=False)
    orgb_d = nc.declare_dram_parameter("orgb", [4, NB * PIX], bf16,
                                       isOutput=True)

    with ExitStack() as ctx:
        tc = ctx.enter_context(tile.TileContext(
            nc, linearize=bool(int(os.environ.get("GR_LINEARIZE", "0")))))
        const_pool = ctx.enter_context(tc.tile_pool(name="const", bufs=1))
        sb = ctx.enter_context(tc.tile_pool(name="work", bufs=2))
        ps = ctx.enter_context(tc.tile_pool(name="psum", bufs=1, space="PSUM"))

        coef_sb = const_pool.tile([16, G * KB], f32r)
        basis_sb = const_pool.tile([16, FD], f32r)
        ucols_sb = const_pool.tile([KB, KB + 4 * NB], bf16)
        out_sb = const_pool.tile([4, NB * PIX], bf16)

        nc.sync.dma_start(coef_sb[:], coef_d[:])
        nc.gpsimd.dma_start(basis_sb[:], basis_d[:])
        nc.gpsimd.dma_start(ucols_sb[:], ucols_d[:])

        u_sb = ucols_sb[:, 0:KB]

        P = [None] * G
        A = [None] * G
        L = [None] * G
        Wt = [None] * G
        R = [None] * G

        def e_pow(i):
            P[i] = ps.tile([KB, FD], f32, tag="p", bufs=3, name=f"P{i}")
            nc.tensor.matmul(P[i], lhsT=coef_sb[:, i * KB:(i + 1) * KB],
                             rhs=basis_sb, start=True, stop=True)

        def e_A(i):
            A[i] = sb.tile([KB, FD], f32, tag="A", name=f"A{i}")
            nc.scalar.activation(A[i], P[i], AF.Exp)

        def e_L(i):
            L[i] = sb.tile([KB, FD], bf16, tag="L", name=f"L{i}")
            nc.scalar.activation(L[i], A[i], AF.Ln, bias=1.0, scale=-1.0)

        def e_u(i):
            nc.tensor.matmul(P[i], lhsT=u_sb, rhs=L[i],
                             start=False, stop=True, skip_group_check=True)

        def e_W(i):
            Wt[i] = sb.tile([KB, FD], bf16, tag="W", name=f"W{i}")
            nc.scalar.activation(Wt[i], P[i], AF.Exp)

        def e_rgb(i):
            R[i] = ps.tile([4, FD], f32, tag="r", bufs=2, name=f"R{i}")
            for k in range(2):
                b = 2 * i + k
                nc.tensor.matmul(
                    R[i][:, k * PIX:(k + 1) * PIX],
                    lhsT=ucols_sb[:, KB + b * 4:KB + (b + 1) * 4],
                    rhs=Wt[i][:, k * PIX:(k + 1) * PIX],
                    start=True, stop=True)

        def e_out(i):
            sl = slice(i * FD, (i + 1) * FD)
            nc.vector.tensor_copy(out_sb[:, sl], R[i])
            nc.sync.dma_start(orgb_d[:, sl], out_sb[:, sl])

        # software-pipelined emission: per-engine program order is emission
        # order; keep the scalar (Act) queue free of stalls.
        e_pow(0)
        e_A(0)
        if G > 1:
            e_pow(1)
        e_L(0)
        e_u(0)
        if G > 1:
            e_A(1)
        if G > 2:
            e_pow(2)
        e_W(0)
        for i in range(1, G):
            # steady state: iter i-1 finishing, iter i mid, iter i+1 starting
            e_L(i)
            e_u(i)
            e_rgb(i - 1)
            e_out(i - 1)
            if i + 1 < G:
                e_A(i + 1)
            if i + 2 < G:
                e_pow(i + 2)
            e_W(i)
        e_rgb(G - 1)
        e_out(G - 1)

    nc.compile()
    return nc


# ----------------------------------------------------------------------------
# Entry point
# ----------------------------------------------------------------------------

def kernel(means3D, means2D, opacities, colors_precomp, scales, rotations,
           bg, viewmatrix):
    import ml_dtypes
    means3D = np.asarray(means3D, np.float32)
    opacities = np.asarray(opacities, np.float32)
    colors_precomp = np.asarray(colors_precomp, np.float32)
    scales = np.asarray(scales, np.float32)
    rotations = np.asarray(rotations, np.float32)
    bg = np.asarray(bg, np.float32)
    viewmatrix = np.asarray(viewmatrix, np.float32)

    pre = _preprocess(means3D, opacities, colors_precomp, scales, rotations,
                      viewmatrix)
    bins = _pack_bins(pre)
    nbins = len(bins)
    NB = max(1, -(-nbins // N_CORES))
    NB += NB % 2          # pad to even (empty bins are all padding rows)
    if bool(int(os.environ.get("GR_DEBUG", "0"))):
        rows = sum(len(s['idx']) for b in bins for s in b)
        print(f"[gr] bins={nbins} NB={NB} rows={rows}")

    core_bins = [bins[c::N_CORES] for c in range(N_CORES)]
    basis = _make_basis()
    basis2 = np.zeros((16, 512), np.float32)
    basis2[0:8, 0:256] = basis
    basis2[8:16, 256:512] = basis
    ustrict = np.triu(np.ones((KB, KB), np.float32), 1)

    in_maps = []
    all_placements = []
    for core in range(N_CORES):
        coef, colsT, placements = _build_core_arrays(pre, core_bins[core], NB)
        all_placements.append(placements)
        # rearrange [8, NB*128] -> [16, (NB/2)*128] (bin pair stacking)
        coef2 = np.zeros((16, (NB // 2) * KB), np.float32)
        for g in range(NB // 2):
            coef2[0:8, g * KB:(g + 1) * KB] = coef[:, (2 * g) * KB:(2 * g + 1) * KB]
            coef2[8:16, g * KB:(g + 1) * KB] = coef[:, (2 * g + 1) * KB:(2 * g + 2) * KB]
        ucols = np.concatenate([ustrict, colsT], axis=1)
        in_maps.append(dict(
            coef2=coef2,
            basis2=basis2,
            ucols=ucols.astype(ml_dtypes.bfloat16)))

    if NB not in _compiled_cache:
        _compiled_cache[NB] = _build_program(NB)
    nc = _compiled_cache[NB]

    from concourse.bass_utils import run_bass_kernel_spmd
    trace = bool(int(os.environ.get("GR_TRACE", "0")))
    res = run_bass_kernel_spmd(nc, in_maps, list(range(N_CORES)), trace=trace)
    if trace:
        kernel.last_exec_time_ns = res.exec_time_ns
        kernel.last_profile = res.profile_json

    # ---- host combine ----
    out = np.zeros((3, H, W), np.float32) + bg[:, None, None]
    chains = {}
    for core in range(N_CORES):
        orgb = np.asarray(res.results[core]["orgb"], np.float32)
        for pl in all_placements[core]:
            c0 = pl['bin'] * PIX + pl['col0']
            rgb = orgb[0:3, c0:c0 + pl['ncols']]
            T = orgb[3, c0:c0 + pl['ncols']]
            chains.setdefault(pl['key'], []).append(
                (pl['pos'], pl['rect'], rgb, T, pl['npos']))
    for key, parts in chains.items():
        parts.sort(key=lambda p: p[0])
        _, rect, rgb0, T0, _ = parts[0]
        acc = rgb0.astype(np.float32).copy()
        Tacc = T0.copy()
        for _, _, rgb, T, _ in parts[1:]:
            acc += Tacc[None, :] * rgb
            Tacc = Tacc * T
        acc += Tacc[None, :] * bg[:, None]
        xlo, ylo, w, hgt = rect
        if w == 16:
            left = acc[:, 0:128].reshape(3, 16, 8)
            right = acc[:, 128:256].reshape(3, 16, 8)
            out[:, ylo:ylo + 16, xlo:xlo + 8] = left
            out[:, ylo:ylo + 16, xlo + 8:xlo + 16] = right
        else:
            out[:, ylo:ylo + hgt, xlo:xlo + w] = acc.reshape(3, hgt, w)
    return out


# revision 5
# speedup vs baseline: 1.3524x; 1.0408x over previous
"""Trainium2 Bass kernel for a 3D-gaussian-splatting rasterizer.

Pipeline:
  host (numpy, O(N) work): quaternion -> cov3D -> EWA cov2D -> conic,
    projection, depth sort, per-cell culling (8x16 half-tiles, with a
    16x16 merge when the halves share most gaussians), packing of
    (cell, gaussian-chunk) slabs into [127-row x 256-col] bins.
  device (8 NeuronCores, SPMD), iteration i = bin pair (2i, 2i+1),
    512 pixel-columns:
      P   = coef-matmul over a 16-term stacked basis (one matmul per
            bin PAIR: rows 0-7 drive cols 0-255, rows 8-15 cols 256-511)
      A   = exp(P)                  (scalar engine)
      L   = ln(1 - A)               (scalar engine, bf16)
      P  += Ustrict @ L             (matmul ACCUMULATED onto the power
            PSUM, start=False -> S = power + cumsum_strict(L))
      W   = exp(S) = alpha * T_excl (scalar engine, bf16) -- the
            compositing weights directly, no elementwise multiply.
      R   = colsT @ W               (per-bin rgb matmul; color column 3
            holds e^{+0.5} at reserved row 127, so R[3] = T_bin because
            W[127] = e^{-0.5} * T_bin)
      copy R -> SBUF (vector engine, bf16), per-iter DMA out on the
      sync engine (hardware DGE; gpsimd software DGE is slow).
    Emission is software-pipelined so the scalar engine (bottleneck)
    never waits on the tensor engine.
  host: combine slabs of multi-chunk cells front-to-back
    (rgb += T_prefix * rgb_slab), add residual transmittance * bg,
    scatter cells into the [3,128,128] image.

No per-element masking is applied (the reference zeroes alpha < 1/255);
numerically validated vs the 2e-2 gate.
"""

import os
import numpy as np

N_CORES = 8
H = W = 128
TS = 16            # tile (full basis pattern) is 16x16
PIX = 256          # columns per bin
ROWCAP = 127       # gaussian rows per bin (row 127 reserved)
KB = 128
TANFOV = 0.5
FOCAL = W / (2.0 * TANFOV)   # 128.0
ZNEAR = 0.2
ALPHA_MIN = 1.0 / 255.0
NEG_BIG = -1.0e9
RESV = -0.5        # reserved-row power; W[127] = e^{RESV} * T_bin

_compiled_cache = {}


# ----------------------------------------------------------------------------
# Host-side per-gaussian preprocessing (numpy, O(N))
# ----------------------------------------------------------------------------

def _preprocess(means3D, opacities, colors_precomp, scales, rotations, viewmatrix):
    q = rotations / np.linalg.norm(rotations, axis=-1, keepdims=True)
    r, x, y, z = q[:, 0], q[:, 1], q[:, 2], q[:, 3]
    R = np.stack([
        1 - 2 * (y * y + z * z), 2 * (x * y - r * z), 2 * (x * z + r * y),
        2 * (x * y + r * z), 1 - 2 * (x * x + z * z), 2 * (y * z - r * x),
        2 * (x * z - r * y), 2 * (y * z + r * x), 1 - 2 * (x * x + y * y),
    ], axis=-1).reshape(-1, 3, 3)
    M = R * scales[:, None, :]
    cov3D = np.einsum('nij,nkj->nik', M, M)

    Wm = viewmatrix[:3, :3]
    t = means3D @ Wm.T + viewmatrix[:3, 3]
    tz = t[:, 2]
    lim = 1.3 * TANFOV
    txz = np.clip(t[:, 0] / tz, -lim, lim) * tz
    tyz = np.clip(t[:, 1] / tz, -lim, lim) * tz
    zero = np.zeros_like(tz)
    fx = fy = FOCAL
    J = np.stack([
        np.stack([fx / tz, zero, -fx * txz / (tz * tz)], axis=-1),
        np.stack([zero, fy / tz, -fy * tyz / (tz * tz)], axis=-1),
    ], axis=1)
    T = np.einsum('nij,jk->nik', J, Wm)
    cov2D = np.einsum('nij,njk,nlk->nil', T, cov3D, T)
    a = cov2D[:, 0, 0] + 0.3
    b = cov2D[:, 0, 1]
    c = cov2D[:, 1, 1] + 0.3
    det = a * c - b * b
    det_safe = np.where(det > 0, det, 1.0)
    conA, conB, conC = c / det_safe, -b / det_safe, a / det_safe
    px = fx * t[:, 0] / tz + (W - 1) * 0.5
    py = fy * t[:, 1] / tz + (H - 1) * 0.5
    valid = (det > 0) & (tz > ZNEAR)
    opac = opacities[:, 0]

    # bounding half-widths of the {alpha >= ALPHA_MIN} ellipse
    ell = np.log(np.maximum(opac * 255.0, 1.0 + 1e-7))
    rx = np.where(valid, np.sqrt(np.maximum(2 * ell * a, 0.0)), 0.0)
    ry = np.where(valid, np.sqrt(np.maximum(2 * ell * c, 0.0)), 0.0)

    order = np.argsort(tz, kind='stable')
    d = dict(conA=conA, conB=conB, conC=conC, px=px, py=py, opac=opac,
             cols=colors_precomp, valid=valid, rx=rx, ry=ry, ell=ell)
    return {k: (v[order] if k != 'cols' else v[order]) for k, v in d.items()}


def _cull_rect(pre, xlo, ylo, w, h):
    """Indices (depth-ordered) of gaussians touching rect, ellipse-corner
    refined."""
    px, py, rx, ry = pre['px'], pre['py'], pre['rx'], pre['ry']
    xhi, yhi = xlo + w - 1, ylo + h - 1
    hit = pre['valid'] & (px + rx >= xlo) & (px - rx <= xhi) \
        & (py + ry >= ylo) & (py - ry <= yhi)
    cx = np.clip(px, xlo, xhi)
    cy = np.clip(py, ylo, yhi)
    dx = cx - px
    dy = cy - py
    beyond = (dx != 0) & (dy != 0)
    quad = pre['conA'] * dx * dx + 2 * pre['conB'] * dx * dy \
        + pre['conC'] * dy * dy
    hit &= ~beyond | (quad <= 2 * pre['ell'])
    return np.nonzero(hit)[0]


def _build_slabs(pre, chunk_cap):
    """Per 16x16 tile choose halves (2 x 8x16) or full tile, chunk lists to
    <=chunk_cap rows, return slab dicts."""
    slabs = []   # dict(rect, idx, key, pos)
    for ti in range(H // TS):
        for tj in range(W // TS):
            xlo, ylo = tj * TS, ti * TS
            idx_l = _cull_rect(pre, xlo, ylo, 8, 16)
            idx_r = _cull_rect(pre, xlo + 8, ylo, 8, 16)
            nl, nr = len(idx_l), len(idx_r)
            if nl == 0 and nr == 0:
                continue
            # full-tile merge when it saves rows and fits one bin
            idx_f = _cull_rect(pre, xlo, ylo, 16, 16)
            nf = len(idx_f)
            use_full = nf <= ROWCAP and nl + nr >= 1.30 * nf and nf > 0
            if use_full:
                slabs.append(dict(rect=(xlo, ylo, 16, 16), idx=idx_f,
                                  key=(ti, tj, 'f'), pos=0, npos=1))
            else:
                for sx, idx in ((0, idx_l), (1, idx_r)):
                    n = len(idx)
                    if n == 0:
                        continue
                    k = -(-n // chunk_cap)
                    chunks = np.array_split(idx, k)
                    for s, ch in enumerate(chunks):
                        slabs.append(dict(rect=(xlo + 8 * sx, ylo, 8, 16),
                                          idx=ch, key=(ti, tj, sx),
                                          pos=s, npos=k))
    return slabs


def _pack_bins_bfd(slabs):
    """Best-fit-decreasing: bins hold <=256 cols (full=256, half=128) and
    <=ROWCAP rows."""
    items = sorted(slabs, key=lambda s: -len(s['idx']))
    bins = []        # list of lists
    space = []       # (cols_left, rows_left)
    for s in items:
        cols = 256 if s['rect'][2] == 16 else 128
        rows = len(s['idx'])
        best, best_slack = -1, None
        for i, (cl, rl) in enumerate(space):
            if cl >= cols and rl >= rows:
                slack = rl - rows
                if best < 0 or slack < best_slack:
                    best, best_slack = i, slack
        if best < 0:
            bins.append([s])
            space.append((256 - cols, ROWCAP - rows))
        else:
            bins[best].append(s)
            cl, rl = space[best]
            space[best] = (cl - cols, rl - rows)
    return bins


def _pack_bins(pre):
    best = None
    for cap in (127, 96, 85, 64):
        slabs = _build_slabs(pre, cap)
        bins = _pack_bins_bfd(slabs)
        if best is None or len(bins) < len(best):
            best = bins
    return best


def _make_basis():
    """[8, 256]: rows x^2,y^2,xy,x,y,1,ind0,ind1. Cols 0-127: left half of a
    16x16 tile (x_rel -7.5..-0.5), 128-255: right half; y-major within."""
    basis = np.zeros((8, 256), np.float32)
    for s in range(2):
        for yy in range(16):
            for xx in range(8):
                c = s * 128 + yy * 8 + xx
                xr = xx + 8 * s - 7.5
                yr = yy - 7.5
                basis[:, c] = [xr * xr, yr * yr, xr * yr, xr, yr, 1.0,
                               1.0 - s, float(s)]
    return basis


def _build_core_arrays(pre, core_bins, NB):
    """coef [8, NB*128] f32, colsT [128, 4*NB] f32 for one core.
    colsT column 3 carries e^{+0.5} at reserved row 127 so the rgb matmul
    emits the bin transmittance in output row 3.
    Returns also slab placement records."""
    coef = np.zeros((8, NB * KB), np.float32)
    colsT = np.zeros((KB, 4 * NB), np.float32)
    # default: padding rows (power = -BIG via const row x basis row5..7)
    coef[5, :] = NEG_BIG
    placements = []
    conA, conB, conC = pre['conA'], pre['conB'], pre['conC']
    px, py, opac, cols = pre['px'], pre['py'], pre['opac'], pre['cols']
    for b, bin_slabs in enumerate(core_bins):
        row = 0
        colbase = 0
        for slab in bin_slabs:
            xlo, ylo, w, hgt = slab['rect']
            idx = slab['idx']
            n = len(idx)
            # slot assignment: full tile uses both slots, half uses one
            if w == 16:
                slot = (0, 1)
                vx = xlo + 7.5
                ncols = 256
            else:
                slot = (colbase // 128,)
                vx = xlo + 7.5 - 8 * (colbase // 128)
                ncols = 128
            vy = ylo + 7.5
            A, Bc, C = conA[idx], conB[idx], conC[idx]
            pxr = px[idx] - vx
            pyr = py[idx] - vy
            sl = slice(b * KB + row, b * KB + row + n)
            coef[0, sl] = -0.5 * A
            coef[1, sl] = -0.5 * C
            coef[2, sl] = -Bc
            coef[3, sl] = A * pxr + Bc * pyr
            coef[4, sl] = C * pyr + Bc * pxr
            coef[5, sl] = -0.5 * (A * pxr * pxr + C * pyr * pyr) \
                - Bc * pxr * pyr + np.log(opac[idx])
            coef[6, sl] = 0.0 if 0 in slot else NEG_BIG
            coef[7, sl] = 0.0 if 1 in slot else NEG_BIG
            colsT[row:row + n, b * 4:b * 4 + 3] = cols[idx]
            placements.append(dict(key=slab['key'], pos=slab['pos'],
                                   npos=slab['npos'], rect=slab['rect'],
                                   bin=b, col0=colbase, ncols=ncols))
            row += n
            colbase += ncols
        # reserved transmittance row
        coef[:, b * KB + ROWCAP] = [0, 0, 0, 0, 0, RESV, 0, 0]
        colsT[ROWCAP, b * 4 + 3] = np.exp(0.5)
    return coef, colsT, placements


# ----------------------------------------------------------------------------
# Device program
# ----------------------------------------------------------------------------

def _build_program(NB):
    """NB must be a multiple of 4. Iteration i covers bins 4i..4i+3 = 1024
    columns (one [128,1024] f32 PSUM tile = 2 banks)."""
    from contextlib import ExitStack
    import concourse.bass as bass  # noqa: F401
    import concourse.tile as tile
    from concourse import mybir, bacc

    f32 = mybir.dt.float32
    f32r = mybir.dt.float32r
    bf16 = mybir.dt.bfloat16
    AF = mybir.ActivationFunctionType

    G = NB // 4          # iterations
    FD = 4 * PIX         # 1024 columns per iteration

    class _BaccOneActSet(bacc.Bacc):
        # Pin Exp/Ln to the one table set containing both, so the scalar
        # engine loads activation tables once.
        def insert_act_table_loads(self):
            from concourse.hw_specs import get_activation_tables
            from concourse.bacc import _bass_rust
            tables = []
            for name, fns in get_activation_tables(self.m.arch).items():
                if name != 'natural_log_exp_and_others':
                    fns = fns - {AF.Exp, AF.Ln}
                tables.append((name, fns))
            _bass_rust.insert_act_table_loads(self, tables)

    nc = _BaccOneActSet(None)
    basis_d = nc.declare_dram_parameter("basis", [8, PIX], f32r,
                                        isOutput=False)
    coef_d = nc.declare_dram_parameter("coef", [8, NB * KB], f32r,
                                       isOutput=False)
    ucols_d = nc.declare_dram_parameter("ucols", [KB, KB + 4 * NB], bf16,
                                        isOutput=False)
    orgb_d = nc.declare_dram_parameter("orgb", [4, NB * PIX], bf16,
                                       isOutput=True)

    with ExitStack() as ctx:
        tc = ctx.enter_context(tile.TileContext(
            nc, linearize=bool(int(os.environ.get("GR_LINEARIZE", "0")))))
        const_pool = ctx.enter_context(tc.tile_pool(name="const", bufs=1))
        sb = ctx.enter_context(tc.tile_pool(name="work", bufs=2))
        ps = ctx.enter_context(tc.tile_pool(name="psum", bufs=1, space="PSUM"))

        basis_sb = const_pool.tile([8, PIX], f32r)
        coef_sb = const_pool.tile([8, NB * KB], f32r)
        ucols_sb = const_pool.tile([KB, KB + 4 * NB], bf16)
        out_sb = const_pool.tile([4, NB * PIX], bf16)

        # input DMAs: small basis first (unblocks the first matmul), then
        # the first iteration's coef slice, then the rest; all on HW-DGE
        # queues (sync + scalar), none on gpsimd's slow software DGE.
        nc.sync.dma_start(basis_sb[:], basis_d[:])
        nc.sync.dma_start(coef_sb[:, 0:4 * KB], coef_d[:, 0:4 * KB])
        nc.sync.dma_start(ucols_sb[:], ucols_d[:])
        if NB > 4:
            nc.scalar.dma_start(coef_sb[:, 4 * KB:], coef_d[:, 4 * KB:])

        u_sb = ucols_sb[:, 0:KB]

        P = [None] * G
        A = [None] * G
        L = [None] * G
        Wt = [None] * G
        R = [None] * G

        def e_pow(i):
            P[i] = ps.tile([KB, FD], f32, tag="p", bufs=2, name=f"P{i}")
            for k in range(4):
                b = 4 * i + k
                # PSUM pending-zero regions are whole 2KB banks: only the
                # FIRST matmul touching a bank may use start=True, or it
                # re-marks the sibling quarter pending and the later
                # accumulating u-matmul would overwrite it.
                first_in_bank = (k % 2 == 0)
                nc.tensor.matmul(P[i][:, k * PIX:(k + 1) * PIX],
                                 lhsT=coef_sb[:, b * KB:(b + 1) * KB],
                                 rhs=basis_sb, start=first_in_bank, stop=True,
                                 skip_group_check=not first_in_bank)

        def e_A(i):
            A[i] = sb.tile([KB, FD], f32, tag="A", name=f"A{i}")
            nc.scalar.activation(A[i], P[i], AF.Exp)

        def e_L(i):
            L[i] = sb.tile([KB, FD], bf16, tag="L", name=f"L{i}")
            nc.scalar.activation(L[i], A[i], AF.Ln, bias=1.0, scale=-1.0)

        def e_u(i):
            for h in range(2):
                sl = slice(h * 512, (h + 1) * 512)
                nc.tensor.matmul(P[i][:, sl], lhsT=u_sb, rhs=L[i][:, sl],
                                 start=False, stop=True,
                                 skip_group_check=True)

        def e_W(i):
            Wt[i] = sb.tile([KB, FD], bf16, tag="W", name=f"W{i}")
            nc.scalar.activation(Wt[i], P[i], AF.Exp)

        def e_rgb(i):
            # per-bin matmul + per-half cast + per-half DMA, staggered so
            # the tail after the last W is short.
            R[i] = ps.tile([4, FD], f32, tag="r", bufs=2, name=f"R{i}")
            for k in range(4):
                b = 4 * i + k
                nc.tensor.matmul(
                    R[i][:, k * PIX:(k + 1) * PIX],
                    lhsT=ucols_sb[:, KB + b * 4:KB + (b + 1) * 4],
                    rhs=Wt[i][:, k * PIX:(k + 1) * PIX],
                    start=True, stop=True)
                if k % 2 == 1:
                    hsl = slice((k - 1) * PIX, (k + 1) * PIX)
                    osl = slice(i * FD + (k - 1) * PIX,
                                i * FD + (k + 1) * PIX)
                    nc.vector.tensor_copy(out_sb[:, osl], R[i][:, hsl])
                    nc.sync.dma_start(orgb_d[:, osl], out_sb[:, osl])

        # software-pipelined emission: per-engine program order is emission
        # order; keep the scalar (Act) queue free of stalls.
        e_pow(0)
        e_A(0)
        if G > 1:
            e_pow(1)
        e_L(0)
        e_u(0)
        if G > 1:
            e_A(1)
        if G > 2:
            e_pow(2)
        e_W(0)
        for i in range(1, G):
            e_L(i)
            e_u(i)
            e_rgb(i - 1)
            if i + 1 < G:
                e_A(i + 1)
            if i + 2 < G:
                e_pow(i + 2)
            e_W(i)
        e_rgb(G - 1)

    nc.compile()
    return nc


# ----------------------------------------------------------------------------
# Entry point
# ----------------------------------------------------------------------------

def kernel(means3D, means2D, opacities, colors_precomp, scales, rotations,
           bg, viewmatrix):
    import ml_dtypes
    means3D = np.asarray(means3D, np.float32)
    opacities = np.asarray(opacities, np.float32)
    colors_precomp = np.asarray(colors_precomp, np.float32)
    scales = np.asarray(scales, np.float32)
    rotations = np.asarray(rotations, np.float32)
    bg = np.asarray(bg, np.float32)
    viewmatrix = np.asarray(viewmatrix, np.float32)

    pre = _preprocess(means3D, opacities, colors_precomp, scales, rotations,
                      viewmatrix)
    bins = _pack_bins(pre)
    nbins = len(bins)
    NB = max(1, -(-nbins // N_CORES))
    NB = -(-NB // 4) * 4  # pad to multiple of 4 (empty bins = padding rows)
    if bool(int(os.environ.get("GR_DEBUG", "0"))):
        rows = sum(len(s['idx']) for b in bins for s in b)
        print(f"[gr] bins={nbins} NB={NB} rows={rows}")

    core_bins = [bins[c::N_CORES] for c in range(N_CORES)]
    basis = _make_basis()
    ustrict = np.triu(np.ones((KB, KB), np.float32), 1)

    in_maps = []
    all_placements = []
    for core in range(N_CORES):
        coef, colsT, placements = _build_core_arrays(pre, core_bins[core], NB)
        all_placements.append(placements)
        ucols = np.concatenate([ustrict, colsT], axis=1)
        in_maps.append(dict(
            coef=coef,
            basis=basis,
            ucols=ucols.astype(ml_dtypes.bfloat16)))

    if NB not in _compiled_cache:
        _compiled_cache[NB] = _build_program(NB)
    nc = _compiled_cache[NB]

    from concourse.bass_utils import run_bass_kernel_spmd
    trace = bool(int(os.environ.get("GR_TRACE", "0")))
    res = run_bass_kernel_spmd(nc, in_maps, list(range(N_CORES)), trace=trace)
    if trace:
        kernel.last_exec_time_ns = res.exec_time_ns
        kernel.last_profile = res.profile_json

    # ---- host combine ----
    out = np.zeros((3, H, W), np.float32) + bg[:, None, None]
    chains = {}
    for core in range(N_CORES):
        orgb = np.asarray(res.results[core]["orgb"], np.float32)
        for pl in all_placements[core]:
            c0 = pl['bin'] * PIX + pl['col0']
            rgb = orgb[0:3, c0:c0 + pl['ncols']]
            T = orgb[3, c0:c0 + pl['ncols']]
            chains.setdefault(pl['key'], []).append(
                (pl['pos'], pl['rect'], rgb, T, pl['npos']))
    for key, parts in chains.items():
        parts.sort(key=lambda p: p[0])
        _, rect, rgb0, T0, _ = parts[0]
        acc = rgb0.astype(np.float32).copy()
        Tacc = T0.copy()
        for _, _, rgb, T, _ in parts[1:]:
            acc += Tacc[None, :] * rgb
            Tacc = Tacc * T
        acc += Tacc[None, :] * bg[:, None]
        xlo, ylo, w, hgt = rect
        if w == 16:
            left = acc[:, 0:128].reshape(3, 16, 8)
            right = acc[:, 128:256].reshape(3, 16, 8)
            out[:, ylo:ylo + 16, xlo:xlo + 8] = left
            out[:, ylo:ylo + 16, xlo + 8:xlo + 16] = right
        else:
            out[:, ylo:ylo + hgt, xlo:xlo + w] = acc.reshape(3, hgt, w)
    return out
